# revision 17
# baseline (speedup 1.0000x reference)
"""Trainium2 Bass kernel for causal multi-head attention.

Reference computation (B=2, T=2048, D=1024, H=16 heads, head_dim=64):
    q, k, v = x @ Wq, x @ Wk, x @ Wv         (per-head split)
    out = softmax(causal(q k^T / 8)) v  @ Wo

Sharding: 8 cores = 2 batches x 4 head-groups (4 heads each).  Each core
computes, for its batch b and its 4 heads:
    qT, kT [256, 2048] and v [2048, 256]  from the host-pre-transposed xT,
    transposed scores sT[tk, tq] = kT.T @ qT  (so softmax sums land on the
    matmul contraction axis and no on-chip transposes are ever needed),
    expS = exp(sT/8) * causal_mask,
    ctxT' [65, tq] = v'.T @ expS   with v' = [v | ones] so row 64 is the
    softmax denominator,
    ctxT_norm = ctxT * (1/rowsum)  (rank-1 PE broadcast of the reciprocal),
    partial_out [2048, 1024] = ctxT.T @ Wo[g*256:(g+1)*256, :].
Host sums the 4 partials per batch.

All matmuls run as float32r (TF32-like, full PE rate at N>=256).  Tiles that
feed the PE are allocated as float32r (walrus requires producer dtype to
match); PSUM accumulation stays fp32.

Host pre-arranges every dram tensor partition-major ([128, ...] with each
partition's data contiguous) so DMAs use 8-16KB descriptors instead of 2KB
ones -- descriptor generation was 2/3 of the old 14us startup stall.

Softmax denominators: one reciprocal_approx_fast [1,512] per head straight
from PSUM (replaces the batched 3.3us DVE reciprocal + memset + casts).

Scheduling: the attention i-loop rotates over a head pair (sT x2 then ctx
x2, one iteration behind, so the PE never waits on an exp).  Fill work is
paced into each half-chunk pass: chunk nj runs Q/K projections of nj+1 plus
norm of nj-1's late heads plus the whole outproj of nj-1 during its first
pass, V projections of nj+1 plus norm of its own first head pair during the
second.  The tail after the last chunk is only norm of two heads + outproj
of the last 4 t-blocks.
"""

import sys

if "/opt/trn_rl_repo" not in sys.path:
    sys.path.insert(0, "/opt/trn_rl_repo")

import numpy as np

B, T, D, H = 2, 2048, 1024, 16
HD = 64                   # head dim
NCORES = 8
GROUPS = 4                # head groups (cores per batch)
HPC = H // GROUPS         # heads per core = 4
DHC = HPC * HD            # per-core head columns = 256
NKB = D // 128            # 8 contraction blocks for the projections
NTB = T // 128            # 16 t-blocks
NCH = T // 512            # 4 tq chunks of 512

_CACHE = {}


def _build():
    import concourse.bacc as bacc
    import concourse.tile as tile
    from concourse import mybir

    # All three activation functions used here (Exp for scores, Ln+Exp for
    # 1/rowsum, Copy for drains) live together in the
    # 'natural_log_exp_and_others' table set, but the table-load placement
    # pass maps each function to the FIRST set containing it, which splits
    # them across sets and inserts a ~2.7us ACT_TABLE_LOAD per switch (21
    # loads).  Hiding Exp/Ln/Copy membership in every other set -- keys and
    # order untouched, so act_func_set_ids stay canonical -- makes the pass
    # place ONE load of the (real, complete) natural_log set.
    _orig_tables = bacc.get_activation_tables
    _strip = {mybir.ActivationFunctionType.Exp,
              mybir.ActivationFunctionType.Ln,
              mybir.ActivationFunctionType.Copy}

    def _one_table(arch):
        tabs = _orig_tables(arch)
        return {k: (set(v) if k == "natural_log_exp_and_others"
                    else set(v) - _strip) for k, v in tabs.items()}

    bacc.get_activation_tables = _one_table

    fp32 = mybir.dt.float32
    bf16 = mybir.dt.bfloat16
    fp32r = mybir.dt.float32r
    Exp = mybir.ActivationFunctionType.Exp
    Ln = mybir.ActivationFunctionType.Ln

    nc = bacc.Bacc("TRN2", target_bir_lowering=False, debug=False,
                   num_devices=NCORES)

    xt_d = nc.dram_tensor("xt", [128, NCH, NKB, 512], bf16,
                          kind="ExternalInput")
    wq_d = nc.dram_tensor("wq", [128, NKB, DHC], bf16, kind="ExternalInput")
    wk_d = nc.dram_tensor("wk", [128, NKB, DHC], bf16, kind="ExternalInput")
    wv_d = nc.dram_tensor("wv", [128, NKB, DHC], bf16, kind="ExternalInput")
    wo_d = nc.dram_tensor("wo", [128, 2, D], bf16, kind="ExternalInput")
    cm_d = nc.dram_tensor("cmask", [128, 1024], bf16, kind="ExternalInput")
    out_d = nc.dram_tensor("out", [T, D], bf16, kind="ExternalOutput")

    with tile.TileContext(nc) as tc:
        with (
            tc.tile_pool(name="consts", bufs=1) as consts,
            tc.tile_pool(name="xtp", bufs=2) as xtp,
            tc.tile_pool(name="big", bufs=1) as big,
            tc.tile_pool(name="es_pool", bufs=8) as es_pool,
            tc.tile_pool(name="small", bufs=3) as small,
            tc.tile_pool(name="outp", bufs=4) as outp,
            tc.tile_pool(name="psum", bufs=1, space="PSUM") as psum,
        ):
            wq_sb = consts.tile([128, NKB, DHC], bf16)
            wk_sb = consts.tile([128, NKB, DHC], bf16)
            wv_sb = consts.tile([128, NKB, DHC], bf16)
            wo_sb = consts.tile([128, 2, D], bf16)
            cm_sb = consts.tile([128, 1024], bf16)
            nc.sync.dma_start(out=wq_sb[:, 0:4, :],
                              in_=wq_d[:, 0:4, :])

            qt_sb = big.tile([128, 2, T], bf16)
            kt_sb = big.tile([128, 2, T], bf16)
            ct_sb = big.tile([128, 2, T], bf16)
            vs_sb = big.tile([128, NTB, HPC, HD + 1], bf16)

            xt_c = [None] * NCH
            # pcS[nj][h]: ctxT' drained to SBUF at end of chunk nj's pass
            pcS = [[None] * HPC for _ in range(NCH)]
            # rcAll[nj][h]: fp32 [1,512] reciprocal of head h's rowsums
            rcAll = [[None] * HPC for _ in range(NCH)]

            def load_xt(nj):
                xt_c[nj] = xtp.tile([128, NKB, 512], bf16, tag="xt",
                                    name=f"xt{nj}")
                half = NKB // 2
                nc.sync.dma_start(out=xt_c[nj][:, 0:half, :],
                                  in_=xt_d[:, nj, 0:half, :])
                nc.sync.dma_start(out=xt_c[nj][:, half:, :],
                                  in_=xt_d[:, nj, half:, :])

            def qkv_halves(nj):
                """16 closures, each half a psum accumulation group (4 MMs).
                First 8 are the Q/K projections, last 8 the V projections."""
                cs = slice(nj * 512, (nj + 1) * 512)
                qk, vq = [], []

                def make_qk(wsb, dst, mb):
                    pq = [None]

                    def go_a():
                        pq[0] = psum.tile([128, 512], fp32, tag="mm", bufs=2,
                                          name=f"pq{nj}{mb}")
                        for kb in range(4):
                            nc.tensor.matmul(
                                pq[0],
                                wsb[:, kb, mb * 128:(mb + 1) * 128],
                                xt_c[nj][:, kb, :],
                                start=(kb == 0), stop=False,
                            )

                    def go_b():
                        for kb in range(4, NKB):
                            nc.tensor.matmul(
                                pq[0],
                                wsb[:, kb, mb * 128:(mb + 1) * 128],
                                xt_c[nj][:, kb, :],
                                start=False, stop=(kb == NKB - 1),
                            )
                        nc.vector.tensor_copy(dst[:, mb, cs], pq[0])
                    return go_a, go_b

                def make_v(tb):
                    pv = [None]

                    def go_a():
                        pv[0] = psum.tile([128, 512], fp32, tag="mm", bufs=2,
                                          name=f"pv{tb}")
                        for kb in range(4):
                            nc.tensor.matmul(
                                pv[0][:, 0:DHC],
                                xt_c[nj][:, kb, (tb - 4 * nj) * 128:(tb - 4 * nj + 1) * 128],
                                wv_sb[:, kb, :],
                                start=(kb == 0), stop=False,
                            )

                    def go_b():
                        for kb in range(4, NKB):
                            nc.tensor.matmul(
                                pv[0][:, 0:DHC],
                                xt_c[nj][:, kb, (tb - 4 * nj) * 128:(tb - 4 * nj + 1) * 128],
                                wv_sb[:, kb, :],
                                start=False, stop=(kb == NKB - 1),
                            )
                        nc.vector.tensor_copy(
                            vs_sb[:, tb, :, 0:HD],
                            pv[0][:, 0:DHC].rearrange("p (h d) -> p h d", h=HPC),
                        )
                    return go_a, go_b

                for mb in range(2):
                    qk.extend(make_qk(wq_sb, qt_sb, mb))
                for mb in range(2):
                    qk.extend(make_qk(wk_sb, kt_sb, mb))
                for tb in range(4 * nj, 4 * nj + 4):
                    vq.extend(make_v(tb))
                return qk, vq

            def norm_fill(nj, h):
                """normalize head h of chunk nj from the SBUF-drained ctxT'."""
                def go():
                    mbh, ro = h >> 1, (h & 1) * 64
                    src = pcS[nj][h]
                    pb = psum.tile([64, 512], fp32, tag="mm", bufs=2,
                                   name=f"pb{nj}{h}")
                    nc.tensor.matmul(pb, cm_sb[0:1, 512:576],
                                     rcAll[nj][h],
                                     start=True, stop=True)
                    nc.vector.tensor_mul(
                        ct_sb[ro:ro + 64, mbh, nj * 512:(nj + 1) * 512],
                        src[0:64, :], pb)
                return go

            def outproj_fill(nj, tb, k):
                """half an output block: columns [k*512, (k+1)*512)."""
                def go():
                    ob = outp.tile([128, 512], bf16, tag="ot",
                                   name=f"ot{tb}{k}")
                    po = psum.tile([128, 512], fp32, tag="mm", bufs=2,
                                   name=f"po{tb}{k}")
                    for mb in range(2):
                        nc.tensor.matmul(
                            po,
                            ct_sb[:, mb, tb * 128:(tb + 1) * 128],
                            wo_sb[:, mb, k * 512:(k + 1) * 512],
                            start=(mb == 0), stop=(mb == 1),
                        )
                    if nj == NCH - 1 and k == 1:
                        # tail: ACT is idle there, halve the drain chain
                        nc.scalar.copy(ob, po)
                    else:
                        nc.vector.tensor_copy(ob, po)
                    nc.sync.dma_start(
                        out=out_d[tb * 128:(tb + 1) * 128,
                                  k * 512:(k + 1) * 512],
                        in_=ob)
                return go

            def outproj_fills(nj):
                return [outproj_fill(nj, tb, k)
                        for tb in range(4 * nj, 4 * nj + 4) for k in range(2)]

            # prologue: wq half + xt0 land first so QKV(0) starts early
            load_xt(0)
            nc.sync.dma_start(out=wq_sb[:, 4:, :],
                              in_=wq_d[:, 4:, :])
            nc.sync.dma_start(out=wk_sb, in_=wk_d[:])
            nc.sync.dma_start(out=wv_sb, in_=wv_d[:])
            load_xt(1)
            nc.sync.dma_start(out=cm_sb, in_=cm_d[:])
            nc.sync.dma_start(out=wo_sb, in_=wo_d[:])
            # ones column of v' (cmask cols 512.. are all 1.0, dtype fp32r)
            nc.vector.tensor_copy(
                vs_sb[:, :, :, 64],
                cm_sb[:, 512:512 + NTB * HPC].rearrange("p (a b) -> p a b", a=NTB),
            )
            qk0, vq0 = qkv_halves(0)
            for q in qk0 + vq0:
                q()

            for nj in range(NCH):
                nb = 4 * nj + 4     # causal: tk-blocks 0 .. nb-1
                if nj + 2 < NCH:
                    load_xt(nj + 2)
                # fill work paced into the two head-pair passes:
                #  pass 0: Q/K proj of nj+1, norm of nj-1's late heads,
                #          outproj of nj-1 (order matters: norm first)
                #  pass 1: V proj of nj+1, norm of this chunk's first pair
                if nj + 1 < NCH:
                    qk, vq = qkv_halves(nj + 1)
                else:
                    qk, vq = [], []
                fills = [list(qk), list(vq)]
                if nj >= 1:
                    fills[0] += [norm_fill(nj - 1, h) for h in (2, 3)]
                if nj >= 2:
                    fills[0] += outproj_fills(nj - 2)
                fills[1] += [norm_fill(nj, h) for h in (0, 1)]
                if nj == NCH - 1:
                    # hp1 of the last chunk has no V-proj fill: park the
                    # previous chunk's outproj here to keep the PE ahead of
                    # the exp stream
                    fills[1] += outproj_fills(nj - 1)

                for hp in range(2):
                    fill = fills[hp]
                    fi = 0
                    heads = (2 * hp, 2 * hp + 1)
                    pcs2 = [psum.tile([65, 512], fp32, tag="acc", bufs=2,
                                      name=f"pc{nj}{h}") for h in heads]
                    prev = None
                    for i in range(nb):
                        m = i - 4 * nj
                        # causal window: diagonal blocks only need cols >= wm
                        # (bf16 runs full rate at any N, so m==3 narrows too)
                        wm = 0 if m < 0 else 128 * m
                        sp = psum.tile([128, 2, 512], fp32, tag="sp",
                                       bufs=2, name=f"sp{nj}{hp}{i}")
                        for k, h in enumerate(heads):
                            mbh, ro = h >> 1, (h & 1) * 64
                            nc.tensor.matmul(
                                sp[:, k, wm:512],
                                kt_sb[ro:ro + 64, mbh, i * 128:(i + 1) * 128],
                                qt_sb[ro:ro + 64, mbh, nj * 512 + wm:(nj + 1) * 512],
                                start=True, stop=True,
                            )
                        # one exp covers the head pair: halves the ACT
                        # instruction count and its per-op PSUM-access tax
                        es = es_pool.tile([128, 2, 512], bf16, tag="es",
                                          bufs=8, name=f"es{nj}{hp}{i}")
                        nc.scalar.activation(out=es[:, :, wm:512],
                                             in_=sp[:, :, wm:512],
                                             func=Exp, scale=0.125)
                        if m >= 0:
                            # only the diagonal 128 cols (plus, for m==3,
                            # the below-window cols) need masking
                            a = wm if m == 3 else 128 * m
                            for k in range(2):
                                nc.vector.tensor_mul(
                                    es[:, k, a:128 * m + 128],
                                    es[:, k, a:128 * m + 128],
                                    cm_sb[:, (3 - m) * 128 + a:512],
                                )
                        cur = es
                        if prev is not None:
                            # ctx for iteration i-1: its exps had a full
                            # iteration of PE work to complete on ACT
                            pes, pwm, pi = prev
                            for k, h in enumerate(heads):
                                nc.tensor.matmul(
                                    pcs2[k][:, pwm:512],
                                    vs_sb[:, pi, h, :],
                                    pes[:, k, pwm:512],
                                    start=(pi == 0), stop=False,
                                )
                        prev = (cur, wm, i)
                        while fi < min(len(fill),
                                       int(len(fill) * (i + 1) / nb + 0.999)):
                            fill[fi]()
                            fi += 1
                    pes, pwm, pi = prev
                    for k, h in enumerate(heads):
                        nc.tensor.matmul(
                            pcs2[k][:, pwm:512],
                            vs_sb[:, pi, h, :],
                            pes[:, k, pwm:512],
                            start=(pi == 0), stop=True,
                        )
                    # drain this pass's ctxT' to SBUF (on ACT) and take the
                    # rowsum reciprocals straight from PSUM (custom DVE op,
                    # ~0.6us; garbage-free: only row 64 is read)
                    for k, h in enumerate(heads):
                        dst = small.tile([64, 512], fp32, tag="pcs", bufs=8,
                                         name=f"pcS{nj}{h}")
                        nc.vector.tensor_copy(dst, pcs2[k][0:64, :])
                        pcS[nj][h] = dst
                        # 1/rowsum as exp(-ln(rowsum)): both on ACT (ln and
                        # exp share one table set), no slow DVE reciprocal,
                        # and the exp legally produces the fp32r the norm
                        # broadcast matmul needs
                        rln = small.tile([1, 512], fp32, tag="rln", bufs=8,
                                         name=f"rln{nj}{h}")
                        nc.scalar.activation(out=rln, in_=pcs2[k][64:65, :],
                                             func=Ln)
                        rc = small.tile([1, 512], bf16, tag="rc", bufs=8,
                                        name=f"rc{nj}{h}")
                        nc.scalar.activation(out=rc, in_=rln, func=Exp,
                                             scale=-1.0)
                        rcAll[nj][h] = rc

                    while fi < len(fill):
                        fill[fi]()
                        fi += 1

            # tail: the last chunk's late heads + its output projection
            for go in [norm_fill(NCH - 1, 2), norm_fill(NCH - 1, 3)] \
                    + outproj_fills(NCH - 1):
                go()

    try:
        nc.compile()
    finally:
        bacc.get_activation_tables = _orig_tables
    return nc


def _causal_mask_block():
    # [128, 1024]: cols 0..383 = 0, cols 384..511 = upper-tri (p <= c-384),
    # cols 512.. = 1.  Slice [(3-m)*128 : (3-m)*128+512] masks a diagonal
    # tk-block at position m within a 512-wide tq chunk.
    from ml_dtypes import bfloat16
    m = np.zeros((128, 1024), np.float32)
    m[:, 512:] = 1.0
    m[:, 384:512] = np.triu(np.ones((128, 128), np.float32))
    return m.astype(bfloat16)


def _prepare_in_maps(x_q, Wq, Wk, Wv, Wo):
    from ml_dtypes import bfloat16
    x_q = np.asarray(x_q, bfloat16)
    Wq = np.asarray(Wq, bfloat16)
    Wk = np.asarray(Wk, bfloat16)
    Wv = np.asarray(Wv, bfloat16)
    Wo = np.asarray(Wo, bfloat16)

    cmask = _causal_mask_block()
    # partition-major layouts (every dram tensor is [128, ...] with each
    # partition's bytes contiguous, so DMA descriptors are 8-16KB):
    #   xt[p, c, k, tt] = x[b].T[k*128+p, c*512+tt]
    xts = [np.ascontiguousarray(
        x_q[b].T.reshape(NKB, 128, NCH, 512).transpose(1, 2, 0, 3))
        for b in range(B)]
    in_maps = []
    for c in range(NCORES):
        b, g = divmod(c, GROUPS)
        sl = slice(g * DHC, (g + 1) * DHC)
        in_maps.append({
            "xt": xts[b],
            "wq": np.ascontiguousarray(
                Wq[:, sl].reshape(NKB, 128, DHC).transpose(1, 0, 2)),
            "wk": np.ascontiguousarray(
                Wk[:, sl].reshape(NKB, 128, DHC).transpose(1, 0, 2)),
            "wv": np.ascontiguousarray(
                Wv[:, sl].reshape(NKB, 128, DHC).transpose(1, 0, 2)),
            "wo": np.ascontiguousarray(
                Wo[sl, :].reshape(2, 128, D).transpose(1, 0, 2)),
            "cmask": cmask,
        })
    return in_maps


def _gather(results):
    out = np.zeros((B, T, D), np.float32)
    for c in range(NCORES):
        out[c // GROUPS] += results[c]["out"].astype(np.float32)
    return out


def get_nc():
    if "nc" not in _CACHE:
        _CACHE["nc"] = _build()
    return _CACHE["nc"]


def kernel(x_q, Wq, Wk, Wv, Wo):
    from concourse.bass_utils import run_bass_kernel_spmd

    nc = get_nc()
    in_maps = _prepare_in_maps(x_q, Wq, Wk, Wv, Wo)
    res = run_bass_kernel_spmd(nc, in_maps, list(range(NCORES)))
    return _gather(res.results)
